# revision 26
# baseline (speedup 1.0000x reference)
"""Causal GQA self-attention (RoPE + QK-RMSNorm) Trainium2 kernel.

Sharding: 8 cores = batch (2) x kv-head-group (4). Each core computes, for
its (batch b, kv-group g): the 4 query heads + 1 kv head of that group,
causal attention over the full sequence, and a partial output projection
y_bg = O_g @ W_O[rows of group g]. Host sums the 4 partials per batch.

Device layout is "transposed" throughout: activations live as [feature,
token] so every matmul contracts over the partition axis with 512-wide
moving operands. Matmuls run in bf16 (f32 PSUM accumulation); softmax,
RoPE and RMS statistics stay f32.

v3 changes vs v2 (506.9us): trace showed the PE idle 185us in ~56 gaps,
each resetting the tensor-engine clock ramp. Root causes fixed here:
 - the softmax-denominator rowsum matmul for the first pair of every
   stream waited on a gpsimd pair-add and head-blocked the in-order PE
   queue ~6-7us per stream. All rowsum matmuls now run at the stream
   tail over DVE pair-sums that had the whole stream to complete.
 - score tiles are single [128,512] PSUM tiles (bufs=3) instead of
   2-bank pairs; diagonal masking uses a narrow add + copy instead of
   memset+full-width add; AV matmuls are split into a mask-independent
   part and a 128-wide diagonal part so the PE never waits on the DVE
   triangle multiply.
 - rope/RMS runs as a 3-deep software pipeline (pre -> add+square ->
   stat+ln+exp -> broadcast+normalize) so the scores matmuls never wait
   on the chain.
 - all inputs are host-pre-arranged into device layout: every DMA is a
   contiguous block, and the first x block is split across the sync and
   vector rings so the first projection starts within ~1us.
 - yout PSUM->SBUF copies moved to the scalar engine (ACT Copy); yout
   and the q-projection get a dedicated PSUM bank so score tiles never
   wait on a yout drain.
"""

import numpy as np
import ml_dtypes

import concourse.bass as bass  # noqa: F401
import concourse.tile as tile
from concourse import bacc, mybir
from concourse import bass_utils

BF16 = mybir.dt.bfloat16
F32 = mybir.dt.float32
NPBF16 = ml_dtypes.bfloat16

P = 128          # partitions == head_dim
HALF = 64        # rope half-dim
TB = 512         # t-block (psum free width)
S = 128          # s-tile (score partition block)
EPS = float(np.finfo(np.float32).eps)

ACT = mybir.ActivationFunctionType


class _one_act_table:
    """Steer Bacc's activation-table chooser to the single set that holds
    every function this kernel uses (Copy/Identity/Square/Ln/Exp), so the
    ScalarE never thrashes ACT_TABLE_LOADs between Ln and Exp."""

    KEEP = "natural_log_exp_and_others"
    FUNCS = None

    def __enter__(self):
        import concourse.hw_specs as hw
        import concourse.bacc as bacc_mod
        if _one_act_table.FUNCS is None:
            _one_act_table.FUNCS = {ACT.Copy, ACT.Identity, ACT.Square,
                                    ACT.Ln, ACT.Exp, ACT.MemsetZero}
        self._orig = hw.get_activation_tables

        def patched(arch):
            tabs = self._orig(arch)
            return {k: (set(s) if k == self.KEEP else set(s) - self.FUNCS)
                    for k, s in tabs.items()}

        hw.get_activation_tables = patched
        bacc_mod.get_activation_tables = patched
        return self

    def __exit__(self, *exc):
        import concourse.hw_specs as hw
        import concourse.bacc as bacc_mod
        hw.get_activation_tables = self._orig
        bacc_mod.get_activation_tables = self._orig
        return False


def _build(T, C, G, n_devices=8):
    """Build the single-core SPMD program. T seq len, C model dim, G q-heads."""
    NB = T // TB         # t-blocks
    NC = C // P          # contraction tiles for projections
    SPB = TB // S        # s-tiles per t-block (4)
    NS = T // S          # s-tiles total
    TPB = TB // P        # t-tiles (128 rows) per block
    NYB = C // TB        # y column blocks

    nc = bacc.Bacc("TRN2", target_bir_lowering=False, debug=False,
                   num_devices=n_devices)

    # all host-pre-arranged to device layout (contiguous DMAs only)
    xB = nc.dram_tensor("xB", [NB, P, NC, TB], BF16, kind="ExternalInput").ap()
    wq = nc.dram_tensor("wq", [G, P, NC, P], BF16, kind="ExternalInput").ap()
    wk = nc.dram_tensor("wk", [P, NC, P], BF16, kind="ExternalInput").ap()
    wv = nc.dram_tensor("wv", [P, NC, P], BF16, kind="ExternalInput").ap()
    wo = nc.dram_tensor("wo", [P, G, C], BF16, kind="ExternalInput").ap()
    ccd = nc.dram_tensor("cc", [P, T], F32, kind="ExternalInput").ap()
    ssd = nc.dram_tensor("ss", [P, T], F32, kind="ExternalInput").ap()
    y = nc.dram_tensor("y", [T, C], BF16, kind="ExternalOutput").ap()

    # strictly-lower-triangle-killing mask for the 128-wide diagonal tile:
    # valid iff p <= f
    pp = np.arange(P)[:, None]
    ff = np.arange(S)[None, :]
    tri_np = np.where(pp <= ff, 1.0, 0.0).astype(NPBF16)
    tri_d = nc.inline_tensor(tri_np, "tri").ap()
    idn_d = nc.inline_tensor(np.eye(P, dtype=NPBF16), "idn").ap()
    onesb_d = nc.inline_tensor(np.ones((P, 1), NPBF16), "onesb").ap()

    with tile.TileContext(nc) as tc:
        with (
            tc.tile_pool(name="const", bufs=1) as const,
            tc.tile_pool(name="resid", bufs=1) as resid,
            tc.tile_pool(name="xp", bufs=2) as xp,
            tc.tile_pool(name="work", bufs=3) as work,
            tc.tile_pool(name="rows", bufs=2) as rows,
            tc.tile_pool(name="pp", bufs=3) as ppool,
            tc.tile_pool(name="yp", bufs=2) as yp,
            # PSUM budget (8 banks): sc x3 + oac x2 + qy x1 + row x2
            tc.tile_pool(name="ps", bufs=1, space="PSUM") as ps,
        ):
            # ---- constants into SBUF, ordered by first use. x block 0 is
            # split across the sync + vector rings; weights go on scalar. ----
            wv_sb = const.tile([P, NC, P], BF16, tag="wv")
            nc.scalar.dma_start(wv_sb[:, 0:4], wv[:, 0:4])
            xs0 = xp.tile([P, NC, TB], BF16, tag="xs", name="xs0")
            nc.sync.dma_start(xs0[:, 0:1], xB[0, :, 0:1])
            wk_sb = const.tile([P, NC, P], BF16, tag="wk")
            nc.scalar.dma_start(wk_sb[:, 0:4], wk[:, 0:4])
            nc.sync.dma_start(xs0[:, 1:3], xB[0, :, 1:3])
            nc.scalar.dma_start(wv_sb[:, 4:NC], wv[:, 4:NC])
            nc.scalar.dma_start(wk_sb[:, 4:NC], wk[:, 4:NC])
            nc.scalar.dma_start(xs0[:, 12:NC], xB[0, :, 12:NC])
            nc.sync.dma_start(xs0[:, 3:8], xB[0, :, 3:8])
            nc.sync.dma_start(xs0[:, 8:12], xB[0, :, 8:12])
            cc_sb = const.tile([P, 2, TB], F32, tag="cc")
            nc.scalar.dma_start(cc_sb[:, 0, :], ccd[:, 0:TB])
            ss_sb = const.tile([P, 2, TB], F32, tag="ss")
            nc.scalar.dma_start(ss_sb[:, 0, :], ssd[:, 0:TB])
            wq_sb = const.tile([P, G, NC, P], BF16, tag="wq")
            for h in range(G):
                nc.scalar.dma_start(wq_sb[:, h], wq[h])
            idn = const.tile([P, P], BF16, tag="idn")
            nc.scalar.dma_start(idn, idn_d)
            ones_b = const.tile([P, 1], BF16, tag="onesb")
            nc.scalar.dma_start(ones_b, onesb_d)
            tri = const.tile([P, S], BF16, tag="tri")
            nc.scalar.dma_start(tri, tri_d)
            wo_sb = const.tile([P, G, C], BF16, tag="wo")
            eps_q = const.tile([P, 1], F32, tag="epsq")
            nc.vector.memset(eps_q, P * EPS)
            eps_k = const.tile([P, 1], F32, tag="epsk")
            nc.vector.memset(eps_k, EPS)

            # ---- resident per-block activations (fine-grained for deps) ----
            qT = [[resid.tile([P, TB], BF16, tag=f"qT{h}_{j}",
                              name=f"qT{h}_{j}") for j in range(NB)]
                  for h in range(G)]
            kT = [resid.tile([P, TB], BF16, tag=f"kT{j}", name=f"kT{j}")
                  for j in range(NB)]
            vN = [resid.tile([P, P], BF16, tag=f"v{si}", name=f"v{si}")
                  for si in range(NS)]
            oT = [[resid.tile([P, TB], BF16, tag=f"oT{h}_{j}",
                              name=f"oT{h}_{j}") for j in range(NB)]
                  for h in range(G)]

            # rope/rms as a 3-deep software pipeline:
            #  pre    : cos/sin multiplies + the half-swap DMA.
            #  flush_a: qr = a + rot (DVE) and the ACT square.
            #  flush_b: stat ones-matmul (PE) + Ln + Exp.
            #  flush_c: partition broadcast (gpsimd) + normalize into the
            #           resident qT/kT tile (DVE).
            pend_a, pend_b, pend_c = [], [], []
            flushed = set()

            def rope_pre(psrc, dest, j, is_q):
                # u = q * [-sin; sin]; rotate_half(u) == rotate_half(q)*[sin; -sin]
                u = work.tile([P, TB], F32, tag="rm", bufs=3)
                nc.vector.tensor_mul(u, psrc, ss_sb[:, j % 2, :])
                a = work.tile([P, TB], F32, tag="ra", bufs=3)
                nc.vector.tensor_mul(a, psrc, cc_sb[:, j % 2, :])
                rot = work.tile([P, TB], F32, tag="rot", bufs=3)
                nc.sync.dma_start(rot[0:HALF, :], u[HALF:P, :])
                nc.sync.dma_start(rot[HALF:P, :], u[0:HALF, :])
                pend_a.append((a, rot, dest, is_q))

            def flush_a():
                if not pend_a:
                    return
                a, rot, dest, is_q = pend_a.pop(0)
                qr = work.tile([P, TB], F32, tag="qr", bufs=3)
                nc.vector.tensor_add(qr, a, rot)
                # square on DVE (right behind the add in the same queue), so
                # the stat matmul never waits behind the ACT exp backlog
                q2 = work.tile([P, TB], BF16, tag="q2", bufs=3, name="q2")
                nc.vector.tensor_mul(q2, qr, qr)
                pend_b.append((qr, q2, dest, is_q))

            def flush_b():
                if not pend_b:
                    return
                qr, q2, dest, is_q = pend_b.pop(0)
                srow = ps.tile([1, TB], F32, tag="row", bufs=2, name="srow")
                nc.tensor.matmul(srow, ones_b, q2, start=True, stop=True)
                # inv = (scale*sum + eps')^-0.5 computed as exp(-0.5*ln(.))
                sq = rows.tile([1, TB], F32, tag="sq")
                if is_q:   # 1/sqrt(sum+d*eps) == rsqrt(mean+eps)/sqrt(d)
                    nc.scalar.activation(sq, srow, ACT.Ln,
                                         bias=eps_q[:1, :], scale=1.0)
                else:
                    nc.scalar.activation(sq, srow, ACT.Ln,
                                         bias=eps_k[:1, :], scale=1.0 / P)
                inv = rows.tile([1, TB], F32, tag="inv")
                nc.scalar.activation(inv, sq, ACT.Exp, scale=-0.5)
                pend_c.append((qr, inv, dest))

            def flush_c():
                if not pend_c:
                    return
                qr, inv, dest = pend_c.pop(0)
                invb = work.tile([P, TB], F32, tag="invb", bufs=2)
                nc.gpsimd.partition_broadcast(invb, inv)
                nc.vector.tensor_mul(dest, qr, invb)
                flushed.add(id(dest))

            def advance():
                flush_c()
                flush_b()
                flush_a()

            def ensure_flushed(dest):
                while id(dest) not in flushed:
                    if pend_c:
                        flush_c()
                    elif pend_b:
                        flush_b()
                    elif pend_a:
                        flush_a()
                    else:
                        raise AssertionError("rope dest never enqueued")

            xs_pref = {}

            def emit_prefetch(jn):
                """Next block's x / cos / sin loads a full block early; x is
                split across the sync and scalar rings so the transfer
                finishes in half the time."""
                blk = slice(jn * TB, (jn + 1) * TB)
                xs = xp.tile([P, NC, TB], BF16, tag="xs")
                nc.sync.dma_start(xs[:, 0:NC // 2], xB[jn, :, 0:NC // 2])
                nc.scalar.dma_start(xs[:, NC // 2:], xB[jn, :, NC // 2:])
                nc.sync.dma_start(cc_sb[:, jn % 2, :], ccd[:, blk])
                nc.sync.dma_start(ss_sb[:, jn % 2, :], ssd[:, blk])
                xs_pref[jn] = xs

            def emit_proj_vk(j):
                """V+K projections interleaved per contraction chunk into two
                single-bank PSUM tiles (block 0 compute chases the x DMA)."""
                xs = xs0 if j == 0 else xs_pref.pop(j)
                psk = ps.tile([P, TB], F32, tag="sc", bufs=3, name="psk")
                psv = ps.tile([P, TB], F32, tag="qy", bufs=1, name="psv")
                for ci in range(NC):
                    nc.tensor.matmul(psv, wv_sb[:, ci, :], xs[:, ci, :],
                                     start=(ci == 0), stop=(ci == NC - 1))
                    nc.tensor.matmul(psk, wk_sb[:, ci, :], xs[:, ci, :],
                                     start=(ci == 0), stop=(ci == NC - 1))
                rope_pre(psk, kT[j], j, False)
                vp = work.tile([P, TB], BF16, tag="vp", bufs=2)
                nc.vector.tensor_copy(vp, psv)
                return xs, vp

            def emit_transposes(j, vp):
                for k4 in range(SPB):
                    pt = ps.tile([P, P], BF16, tag="sc", bufs=3, name="pt")
                    nc.tensor.transpose(pt, vp[:, k4 * P:(k4 + 1) * P], idn)
                    nc.vector.tensor_copy(vN[j * SPB + k4], pt)

            def emit_proj_q(j, h, xs):
                qps = ps.tile([P, TB], F32, tag="qy", bufs=1, name="qps")
                for ci in range(NC):
                    nc.tensor.matmul(qps, wq_sb[:, h, ci, :], xs[:, ci, :],
                                     start=(ci == 0), stop=(ci == NC - 1))
                advance()
                rope_pre(qps, qT[h][j], j, True)

            def emit_yout_tile(jb, ti, yb, ys, tag="qy", bufs=1, tail=False):
                """One [128,512] tile of the output projection for token-row
                ti; DMAs the full 128-token stripe once its last column block
                is done. PSUM->SBUF copy on ACT (DVE is busier); at the tail
                the copies and stripe DMAs rotate engines so the post-matmul
                drain isn't serialized on one queue."""
                yps = ps.tile([P, TB], F32, tag=tag, bufs=bufs, name="yps")
                for g in range(G):
                    nc.tensor.matmul(
                        yps,
                        oT[g][jb][:, (ti % TPB) * P:(ti % TPB + 1) * P],
                        wo_sb[:, g, yb * TB:(yb + 1) * TB],
                        start=(g == 0), stop=(g == G - 1))
                # gpsimd can't read PSUM, so copies alternate DVE / ACT
                if yb % 2 == 0:
                    nc.vector.tensor_copy(ys[:, yb, :], yps)
                else:
                    nc.scalar.activation(ys[:, yb, :], yps, ACT.Copy)
                if yb == NYB - 1:
                    q = nc.scalar if (tail and ti % 2) else nc.sync
                    q.dma_start(
                        y[ti * P:(ti + 1) * P, :].rearrange(
                            "p (yb t) -> p yb t", yb=NYB), ys)

            def emit_attn_head(j, h, fillers=()):
                """Causal attention stream (scores -> exp -> AV) for head h of
                t-block j. Rowsum matmuls run at the stream tail so they never
                head-block the PE queue; `fillers` are zero-arg emit thunks of
                independent PE work woven between pairs."""
                advance()
                ensure_flushed(kT[j])
                ensure_flushed(qT[h][j])
                fillers = list(fillers)
                ns = (j + 1) * SPB
                npair = ns // 2
                oac = ps.tile([P, TB], F32, tag="oac", bufs=2, name="oac")
                pexs = [None] * ns
                psum2s = []

                def emit_score(si):
                    """One score tile: matmul, exp, diagonal triangle mask."""
                    if si >= ns:
                        return
                    o = si - j * SPB
                    w = S * o if o > 0 else 0
                    sps = ps.tile([P, TB], F32, tag="sc", bufs=3, name="sps")
                    nc.tensor.matmul(
                        sps[:, w:TB],
                        kT[si // SPB][:, (si % SPB) * S:(si % SPB + 1) * S],
                        qT[h][j][:, w:TB], start=True, stop=True)
                    pex = ppool.tile([P, TB], BF16, tag="p", bufs=4,
                                     name="pex")
                    nc.scalar.activation(pex[:, w:TB], sps[:, w:TB], ACT.Exp)
                    if o >= 0:
                        nc.vector.tensor_mul(pex[:, S * o:S * (o + 1)],
                                             pex[:, S * o:S * (o + 1)], tri)
                    pexs[si] = (pex, w, o)

                # scores run two tiles ahead of AV so the PE never waits on
                # the exp of the tile it is about to consume.
                emit_score(0)
                emit_score(1)
                last = ns - 1
                rrow = ps.tile([1, TB], F32, tag="row", bufs=2, name="rrow")

                def emit_rowsum(a):
                    p2, w0 = psum2s[a]
                    nc.tensor.matmul(rrow[:, w0:TB], ones_b, p2[:, w0:TB],
                                     start=(a == 0), stop=(a == npair - 1))

                for a in range(npair):
                    emit_score(2 * a + 2)
                    emit_score(2 * a + 3)
                    for i in (2 * a, 2 * a + 1):
                        pex, w, o = pexs[i]
                        if o >= 0 and i > 0:
                            # mask-independent part first (no DVE dep), then
                            # the 128-wide diagonal part (tri-mul'd). Only one
                            # start=True write per bank is allowed, so tile 0
                            # (j==0 streams) stays unsplit below.
                            if o < SPB - 1:
                                nc.tensor.matmul(
                                    oac[:, S * (o + 1):TB], vN[i],
                                    pex[:, S * (o + 1):TB],
                                    start=False, stop=False)
                            nc.tensor.matmul(
                                oac[:, S * o:S * (o + 1)], vN[i],
                                pex[:, S * o:S * (o + 1)],
                                start=False, stop=(i == last))
                        else:
                            nc.tensor.matmul(oac[:, w:TB], vN[i],
                                             pex[:, w:TB],
                                             start=(i == 0), stop=False)
                    (pex0, w0, _), (pex1, w1, _) = pexs[2 * a], pexs[2 * a + 1]
                    psum2 = ppool.tile([P, TB], BF16, tag="p2", bufs=8,
                                       name="psum2")
                    if w1 > w0:
                        nc.vector.tensor_add(psum2[:, w1:TB], pex0[:, w1:TB],
                                             pex1[:, w1:TB])
                        nc.vector.tensor_copy(psum2[:, w0:w1], pex0[:, w0:w1])
                    else:
                        nc.vector.tensor_add(psum2, pex0, pex1)
                    psum2s.append((psum2, w0))
                    pexs[2 * a] = pexs[2 * a + 1] = None
                    # rowsums trail the AV loop by two pairs so their DVE
                    # pair-sum has ~2us to land, yet they stay spread out
                    # instead of clustering into a tail stall
                    if a >= 2:
                        emit_rowsum(a - 2)
                    if fillers:
                        fillers.pop(0)()
                # ---- stream tail: denominator + normalize ----
                for a in range(max(npair - 2, 0), npair):
                    emit_rowsum(a)
                rln = rows.tile([1, TB], F32, tag="rln")
                nc.scalar.activation(rln, rrow, ACT.Ln)
                rinv = rows.tile([1, TB], F32, tag="rinv")
                nc.scalar.activation(rinv, rln, ACT.Exp, scale=-1.0)
                rb = work.tile([P, TB], F32, tag="rb", bufs=2)
                nc.gpsimd.partition_broadcast(rb, rinv)
                nc.vector.tensor_mul(oT[h][j], oac, rb)
                for f in fillers:
                    f()

            def yout_fillers(jb, h, ys_box, tag="qy", bufs=1, tail=False):
                """Thunks for the 4 column blocks of token-row jb*TPB+h."""
                ti = jb * TPB + h

                def mk(yb):
                    def f():
                        if yb == 0:
                            ys_box[0] = yp.tile([P, NYB, TB], BF16, tag="ys",
                                                name="ys")
                        emit_yout_tile(jb, ti, yb, ys_box[0], tag, bufs, tail)
                    return f
                return [mk(yb) for yb in range(NYB)]

            # interleave: projections of block j run alongside attention of
            # block j-1 and the output projection of block j-2; yout tiles
            # are woven between attention pairs. Each attention stream is
            # emitted BEFORE the same head's Q projection so the DVE queue
            # is never stuck behind rope multiplies.
            for j in range(NB):
                xs, vp = emit_proj_vk(j)
                if j == 1:
                    # block-0 streams are tiny: run all four before the q
                    # projections so consecutive projections never fight for
                    # the qy bank with only a 1.7us stream between them
                    for h in range(G):
                        emit_attn_head(0, h)
                    emit_transposes(1, vp)
                    for h in range(G):
                        emit_proj_q(1, h, xs)
                    if j + 1 < NB:
                        emit_prefetch(j + 1)
                    continue
                for h in range(G):
                    if j >= 2:
                        emit_attn_head(j - 1, h, yout_fillers(j - 2, h, [None]))
                    if h == 2:
                        emit_transposes(j, vp)
                    emit_proj_q(j, h, xs)
                if j == 0:
                    nc.scalar.dma_start(wo_sb, wo)
                    # drain the whole rope pipeline so iteration 1's streams
                    # never wait on a block-0 normalize chain
                    while pend_a or pend_b or pend_c:
                        advance()
                if j + 1 < NB:
                    emit_prefetch(j + 1)
            # tail: last block's attention woven with block NB-2's output
            # projection, then the final block's output projection (rotating
            # through the now-idle score banks so the copies overlap).
            for h in range(G):
                emit_attn_head(NB - 1, h,
                               yout_fillers(NB - 2, h, [None])
                               if NB >= 2 else ())
            for h in range(G):
                ys_box = [None]
                for f in yout_fillers(NB - 1, h, ys_box, tag="sc", bufs=3,
                                      tail=True):
                    f()

    with _one_act_table():
        nc.compile()
    return nc


_NC_CACHE = {}


def _get_nc(T, C, G):
    key = (T, C, G)
    if key not in _NC_CACHE:
        _NC_CACHE[key] = _build(T, C, G)
    return _NC_CACHE[key]


def _host_prep(x, cos, sin, W_Q, W_K, W_V, W_O, G):
    """Build the 8 per-core input maps (batch-major, then kv-group), with
    every tensor pre-arranged into the exact device SBUF layout."""
    B, T, C = x.shape
    n_kv = W_K.shape[1] // P
    NB = T // TB
    NC = C // P
    cosT = np.ascontiguousarray(cos.reshape(T, HALF).T.astype(np.float32))
    sinT = np.ascontiguousarray(sin.reshape(T, HALF).T.astype(np.float32))
    cc = np.concatenate([cosT, cosT], axis=0)            # [128, T]
    ss = np.concatenate([-sinT, sinT], axis=0)           # [128, T]
    W_Q = np.asarray(W_Q, dtype=np.float32)
    W_K = np.asarray(W_K, dtype=np.float32)
    W_V = np.asarray(W_V, dtype=np.float32)
    W_O = np.asarray(W_O, dtype=np.float32)
    xBs = []
    for b in range(B):
        # [NB, P, NC, TB]: xB[j, p, ci, t] = x[b, j*TB+t, ci*P+p]
        xb = np.ascontiguousarray(
            x[b].reshape(NB, TB, NC, P).transpose(0, 3, 2, 1)).astype(NPBF16)
        xBs.append(xb)
    in_maps = []
    for b in range(B):
        for g in range(n_kv):
            Wqg = W_Q[:, g * G * P:(g + 1) * G * P]
            wq = np.ascontiguousarray(
                Wqg.reshape(NC, P, G, P).transpose(2, 1, 0, 3)).astype(NPBF16)
            wk = np.ascontiguousarray(
                W_K[:, g * P:(g + 1) * P].reshape(NC, P, P)
                .transpose(1, 0, 2)).astype(NPBF16)
            wv = np.ascontiguousarray(
                W_V[:, g * P:(g + 1) * P].reshape(NC, P, P)
                .transpose(1, 0, 2)).astype(NPBF16)
            wo = np.ascontiguousarray(
                W_O[g * G * P:(g + 1) * G * P, :].reshape(G, P, C)
                .transpose(1, 0, 2)).astype(NPBF16)
            in_maps.append({
                "xB": xBs[b], "wq": wq, "wk": wk, "wv": wv, "wo": wo,
                "cc": cc, "ss": ss,
            })
    return in_maps


def kernel(x, cos, sin, W_Q, W_K, W_V, W_O):
    B, T, C = x.shape
    n_kv = W_K.shape[1] // P
    n_head = W_Q.shape[1] // P
    G = n_head // n_kv
    x = np.asarray(x, dtype=np.float32)
    nc = _get_nc(T, C, G)
    in_maps = _host_prep(x, np.asarray(cos), np.asarray(sin),
                         np.asarray(W_Q), np.asarray(W_K), np.asarray(W_V),
                         np.asarray(W_O), G)
    res = bass_utils.run_bass_kernel_spmd(
        nc, in_maps, core_ids=list(range(B * n_kv)))
    out = np.zeros((B, T, C), dtype=np.float32)
    for b in range(B):
        for g in range(n_kv):
            out[b] += np.asarray(res.results[b * n_kv + g]["y"],
                                 dtype=np.float32)
    return out
